# revision 17
# baseline (speedup 1.0000x reference)
"""Causal self-attention (B=2, T=4096, C=768, H=12) on 8 trn2 NeuronCores.

Sharding: core c handles batch b = c//4 and the 3 heads of head-group
hg = c%4 (tensor parallel over heads, data parallel over batch).  Each core
computes the qkv projection for its heads, causal attention, and a partial
output projection; the host sums the 4 per-head-group partials per batch.

Device notes:
  - Matmul inputs are bf16 (fp32 matmul runs LOW_HIGH = 2 PE passes);
    accumulation is fp32 in PSUM.  Host pre-transposes all operands so the
    contraction dim is on SBUF partitions.
  - Scores are computed transposed (S^T[tk, tq] = K Q^T) so P V needs no
    on-chip transposes.  The d=64 contraction runs in the 64x128 row-tiled
    PE mode (two heads share the pair structure, though execution is
    sequential per row tile).
  - Single fused pipeline: the qkv projection for chunk qc+1 and the output
    projection for chunk qc-1 are emitted as PE "filler" groups between the
    S^T score pairs and PV groups of chunk qc, so the Tensor engine never
    drains (keeping its DVFS p-state at max) while the ACT engine - the
    true bottleneck at ~1 exp elem/cycle/partition - runs saturated from
    ~6us into the kernel.
  - Causal trim: on diagonal k-blocks only q >= kb*128 columns are computed
    by S^T, exp'd, masked, and consumed by PV (the strictly-masked triangle
    is never touched; PV reads a restricted column range so stale SBUF is
    never consumed).
  - Softmax denominator comes from an all-ones 65th column appended to V;
    normalization broadcasts the reciprocal row across partitions on GpSimd.
    Softmax skips the max subtraction: scores are ~N(0,1), exp is fp32-safe.
"""

import ml_dtypes
import numpy as np

import concourse.bass as bass
import concourse.mybir as mybir
import concourse.tile as tile
from concourse import bacc

B, T, C, H, HD = 2, 4096, 768, 12, 64
F32 = mybir.dt.float32
BF16 = mybir.dt.bfloat16
N_CORES = 8
AF = mybir.ActivationFunctionType


def build_nc(seq_len: int = T) -> bass.Bass:
    assert seq_len % 512 == 0
    TCH = seq_len // 512   # 512-wide t-chunks
    TB = seq_len // 128    # 128-wide t-blocks

    nc = bacc.Bacc(num_devices=N_CORES)

    xT = nc.dram_tensor("xT", (C, seq_len), BF16, kind="ExternalInput").ap()
    wqkT = nc.dram_tensor("wqkT", (C, 384), BF16, kind="ExternalInput").ap()
    wvT = nc.dram_tensor("wvT", (C, 192), BF16, kind="ExternalInput").ap()
    wpT = nc.dram_tensor("wpT", (192, C), BF16, kind="ExternalInput").ap()
    out = nc.dram_tensor("out", (seq_len, C), F32, kind="ExternalOutput").ap()

    with tile.TileContext(nc) as tc:
        with (
            tc.tile_pool(name="const", bufs=1) as const,
            tc.tile_pool(name="persist", bufs=1) as persist,
            tc.tile_pool(name="xt", bufs=3) as xtpool,
            tc.tile_pool(name="qt", bufs=3) as qtpool,
            tc.tile_pool(name="ot", bufs=3) as otpool,
            tc.tile_pool(name="p", bufs=48) as ppool,
            tc.tile_pool(name="small", bufs=4) as spool,
            tc.tile_pool(name="osb", bufs=3) as osbpool,
            tc.tile_pool(name="ps", bufs=2, space="PSUM") as pspool,
            tc.tile_pool(name="qp", bufs=2, space="PSUM") as qppool,
            tc.tile_pool(name="pso", bufs=2, space="PSUM") as psopool,
        ):
            # ---- constants / weights ----
            # Startup-critical DMAs: per-cc block transfers (contiguous rows,
            # no gather) spread across FOUR engine queues so the ~600ns
            # per-descriptor issue cost parallelizes; the first qkv matmul
            # only waits on wqk[cc0] + xt0[cc0].
            wqk_sb = const.tile([128, 6, 384], BF16, tag="wqk")
            xt0 = xtpool.tile([128, 6, 512], BF16, tag="xt", name="xt")
            wv_sb = const.tile([128, 6, 192], BF16, tag="wv")
            for cc in range(6):
                nc.sync.dma_start(wqk_sb[:, cc, :], wqkT[cc * 128:(cc + 1) * 128, :])
                nc.gpsimd.dma_start(xt0[:, cc, :], xT[cc * 128:(cc + 1) * 128, 0:512])
                nc.scalar.dma_start(wv_sb[:, cc, :], wvT[cc * 128:(cc + 1) * 128, :])
            wp0_sb = const.tile([128, 768], BF16, tag="wp0")
            nc.scalar.dma_start(wp0_sb, wpT[0:128, :])
            # zero-pad wp1 to 128 partitions so the proj matmul stays K=128
            # (avoids a 64x128 <-> 128x128 PE mode switch per t-block)
            wp1_sb = const.tile([128, 768], BF16, tag="wp1")
            nc.vector.memset(wp1_sb[64:128, :], 0.0)
            nc.scalar.dma_start(wp1_sb[0:64, :], wpT[128:192, :])

            # emask[i, qw] = 1.0 if qw >= i else 0.0 (staircase for the one
            # 128x128 block straddling the causal diagonal)
            emask = const.tile([128, 128], BF16, tag="emask")
            nc.gpsimd.memset(emask, 1.0)
            nc.gpsimd.affine_select(
                out=emask, in_=emask,
                compare_op=mybir.AluOpType.is_ge,
                fill=0.0, base=0, pattern=[[1, 128]], channel_multiplier=-1,
            )

            # ---- persistent activations ----
            # kT slab0: h0 @ partitions 0-63, h1 @ 64-127.
            # slab1: h2 duplicated to both halves (enables the 2-sided pair
            # structure with h2 paired against itself on two k-ranges).
            kT_sb = persist.tile([128, 2, seq_len], BF16, tag="kT")
            # v per head: [t-partition, kb, 64 dims + ones column]
            v_sb = [
                persist.tile([128, TB, 65], BF16, tag=f"v{h}", name=f"v{h}")
                for h in range(3)
            ]
            for h in range(3):
                nc.gpsimd.memset(v_sb[h][:, :, 64], 1.0)

            # per-chunk ring tiles (q / attention-out live one chunk only)
            qt_tiles: dict[int, object] = {}
            ot_tiles: dict[int, object] = {}

            # ---- qkv projection filler groups ----
            def make_qkv_fillers(tci, xt_pre=None):
                tcs = slice(tci * 512, (tci + 1) * 512)
                if xt_pre is not None:
                    xt = xt_pre
                else:
                    xt = xtpool.tile([128, 6, 512], BF16, tag="xt", name="xt")
                    for cc in range(6):
                        nc.gpsimd.dma_start(
                            xt[:, cc, :], xT[cc * 128:(cc + 1) * 128, tcs]
                        )
                fs = []

                # q/k channels: m0=[q_h0|q_h1], m1=[k_h0|k_h1], m2=[q_h2|k_h2]
                def gm(m, xt=xt, tci=tci, tcs=tcs):
                    ps = qppool.tile([128, 512], F32, tag="qp", name="ps")
                    for cc in range(6):
                        nc.tensor.matmul(
                            ps,
                            lhsT=wqk_sb[:, cc, m * 128:(m + 1) * 128],
                            rhs=xt[:, cc, :],
                            start=(cc == 0), stop=(cc == 5),
                        )
                    if m == 0:
                        qt = qtpool.tile([128, 2, 512], BF16, tag="qt", name="qt")
                        qt_tiles[tci] = qt
                        nc.vector.tensor_copy(qt[:, 0, :], ps)
                    elif m == 1:
                        nc.vector.tensor_copy(kT_sb[:, 0, tcs], ps)
                    else:
                        # h2: land q at 0-63 / k at 64-127, then duplicate to
                        # the opposite half via SBUF->SBUF DMA.
                        qt = qt_tiles[tci]
                        nc.vector.tensor_copy(qt[0:64, 1, :], ps[0:64, :])
                        nc.vector.tensor_copy(kT_sb[64:128, 1, tcs], ps[64:128, :])
                        nc.sync.dma_start(qt[64:128, 1, :], qt[0:64, 1, :])
                        nc.sync.dma_start(kT_sb[0:64, 1, tcs], kT_sb[64:128, 1, tcs])

                for m in range(3):
                    fs.append(lambda m=m: gm(m))

                # v channels
                def gv(tb, xt=xt, tci=tci):
                    psv = qppool.tile([128, 512], F32, tag="qp", name="psv")
                    for cc in range(6):
                        nc.tensor.matmul(
                            psv[:, :192],
                            lhsT=xt[:, cc, tb * 128:(tb + 1) * 128],
                            rhs=wv_sb[:, cc, :],
                            start=(cc == 0), stop=(cc == 5),
                        )
                    for h in range(3):
                        nc.vector.tensor_copy(
                            v_sb[h][:, tci * 4 + tb, 0:64],
                            psv[:, 64 * h:64 * h + 64],
                        )

                for tb in range(4):
                    fs.append(lambda tb=tb: gv(tb))
                return fs

            # ---- output projection filler groups (chunk qc) ----
            def make_proj_fillers(qc):
                def gp(tbl, qc=qc):
                    tbs = slice((4 * qc + tbl) * 128, (4 * qc + tbl + 1) * 128)
                    lbs = slice(tbl * 128, (tbl + 1) * 128)
                    ot = ot_tiles[qc]
                    ob = osbpool.tile([128, 768], F32, tag="osb")
                    for n0, nsz in ((0, 512), (512, 256)):
                        pp = qppool.tile([128, 512], F32, tag="qp", name="pp")
                        nc.tensor.matmul(
                            pp[:, :nsz],
                            lhsT=ot[:, 0, lbs],
                            rhs=wp0_sb[:, n0:n0 + nsz],
                            start=True, stop=False,
                        )
                        nc.tensor.matmul(
                            pp[:, :nsz],
                            lhsT=ot[:, 1, lbs],
                            rhs=wp1_sb[:, n0:n0 + nsz],
                            start=False, stop=True,
                        )
                        nc.vector.tensor_copy(ob[:, n0:n0 + nsz], pp[:, :nsz])
                    nc.sync.dma_start(out[tbs, :], ob)

                return [lambda tbl=tbl: gp(tbl) for tbl in range(4)]

            # ---- filler scheduling ----
            fillers: list = []

            class Slots:
                left = 1

            def pop_fillers():
                if not fillers:
                    return
                n = -(-len(fillers) // max(Slots.left, 1))
                for _ in range(min(n, len(fillers))):
                    fillers.pop(0)()

            # ---- attention chunk pieces ----
            def st_pair(sides, qc, ptmap):
                # sides: (head, slab, base, kb0) x2 -> one [128,1024] psum
                # per side holding k-blocks kb0, kb0+1.  On diagonal k-blocks
                # only q >= poff*128 is computed / exp'd / masked.
                sps = [
                    pspool.tile([128, 1024], F32, tag="ps", name="sp")
                    for _ in sides
                ]
                qt = qt_tiles[qc]
                for t in (0, 1):
                    for (h, slab, base, kb0), sp in zip(sides, sps):
                        kb = kb0 + t
                        qstart = max(0, (kb - 4 * qc) * 128)
                        nc.tensor.matmul(
                            sp[:, t * 512 + qstart:(t + 1) * 512],
                            lhsT=kT_sb[base:base + 64, slab,
                                       kb * 128:(kb + 1) * 128],
                            rhs=qt[base:base + 64, slab, qstart:512],
                            start=True, stop=True,
                        )
                for (h, slab, base, kb0), sp in zip(sides, sps):
                    pt = ppool.tile([128, 1024], BF16, tag="p", name="pt")
                    poffs = [kb0 + t - 4 * qc for t in (0, 1)]
                    if all(p < 0 for p in poffs):
                        nc.scalar.activation(pt, sp, AF.Exp, scale=0.125)
                    else:
                        for t in (0, 1):
                            qstart = max(0, poffs[t]) * 128
                            rng = slice(t * 512 + qstart, (t + 1) * 512)
                            nc.scalar.activation(
                                pt[:, rng], sp[:, rng], AF.Exp, scale=0.125
                            )
                    for t in (0, 1):
                        poff = poffs[t]
                        if poff >= 0:  # block straddling the causal diagonal
                            rng = slice(t * 512 + poff * 128,
                                        t * 512 + (poff + 1) * 128)
                            nc.vector.tensor_mul(pt[:, rng], pt[:, rng], emask)
                        ptmap[(h, kb0 + t)] = (pt, t)

            # ---- PV pieces (d-major) + normalization ----
            # pso accumulates [65d (incl denominator row), 512q] per head;
            # emission is split into 4-kb "pieces" so they interleave with
            # S^T pairs (keeping the PE busy while ACT drains the exp queue,
            # which holds the DVFS p-state at max).
            pso_tiles: dict = {}

            def pv_piece(h, qc, kb0, nkb, ptmap):
                if kb0 == 0:
                    pso_tiles[h] = psopool.tile(
                        [128, 512], F32, tag="pso", name="pso"
                    )
                pso = pso_tiles[h]
                for kb in range(kb0, min(kb0 + 4, nkb)):
                    pt, t = ptmap[(h, kb)]
                    qstart = max(0, (kb - 4 * qc) * 128)
                    nc.tensor.matmul(
                        pso[0:65, qstart:512],
                        lhsT=v_sb[h][:, kb, :],
                        rhs=pt[:, t * 512 + qstart:(t + 1) * 512],
                        start=(kb == 0), stop=(kb == nkb - 1),
                        skip_group_check=True,
                    )
                if kb0 + 4 >= nkb:
                    # evacuate PSUM so the bank frees quickly
                    ocp = spool.tile([128, 512], F32, tag="ocp")
                    nc.vector.tensor_copy(ocp[0:65, :], pso[0:65, :])
                    return ocp
                return None

            def norm_head(h, slab, qc, ocp):
                # reciprocal of the 512-wide sums row: spread it over 64
                # partitions via SBUF DMA so the iterative DVE reciprocal
                # runs ~64x faster than on a single-partition row
                ot = ot_tiles[qc]
                lsplit = spool.tile([64, 8], F32, tag="lsplit")
                nc.sync.dma_start(lsplit, ocp[64:65, :])
                lrec = spool.tile([64, 8], F32, tag="lrec")
                nc.vector.reciprocal(lrec, lsplit)
                lrow = spool.tile([1, 512], F32, tag="lrow")
                nc.sync.dma_start(lrow, lrec)
                bc = spool.tile([64, 512], F32, tag="bc")
                nc.gpsimd.partition_broadcast(bc, lrow)
                if h == 1:
                    stg2 = spool.tile([64, 512], BF16, tag="stg2")
                    nc.vector.tensor_mul(stg2, ocp[0:64, :], bc)
                    nc.sync.dma_start(ot[64:128, 0, :], stg2)
                else:
                    nc.vector.tensor_mul(ot[0:64, slab, :], ocp[0:64, :], bc)

            # ---- fused pipeline over q-chunks ----
            f0 = make_qkv_fillers(0, xt_pre=xt0)
            for f in f0[:3]:
                f()                      # q/k for chunk 0 inline (PE ramp)
            fillers.extend(f0[3:])       # v-groups of chunk 0
            if TCH > 1:
                fillers.extend(make_qkv_fillers(1))

            for qc in range(TCH):
                nkb = 4 * (qc + 1)
                half = nkb // 2
                ptmap: dict = {}

                if qc + 2 <= TCH - 1:
                    fillers.extend(make_qkv_fillers(qc + 2))
                if qc >= 1:
                    fillers.extend(make_proj_fillers(qc - 1))

                h01_pairs = [
                    ((0, 0, 0, kbp), (1, 0, 64, kbp))
                    for kbp in range(0, nkb, 2)
                ]
                h2_pairs = [
                    ((2, 1, 0, j), (2, 1, 64, half + j))
                    for j in range(0, half, 2)
                ]
                # attention output, transposed: slab0 = [h0|h1], slab1 = [h2|0]
                ot = otpool.tile([128, 2, 512], BF16, tag="ot", name="ot")
                ot_tiles[qc] = ot
                # wp1 rows 64-127 are zero so slab1's lower half is never
                # read with nonzero weight, but stale SBUF could hold NaN
                # patterns; keep it zeroed.
                nc.gpsimd.memset(ot[64:128, 1, :], 0.0)

                # piece queues: one PV piece lands after (almost) every
                # S^T pair so PE work paces the ACT exp drain (pair 852ns PE
                # feeds 1706ns ACT; pair+piece is ~1704ns PE).  Lag 3 pairs
                # so a piece's pt tiles are exp'd ~1.7us before it issues.
                qA = [(h, 4 * j) for j in range(qc + 1) for h in (0, 1)]
                qB = [(2, 4 * j) for j in range(qc + 1)]
                ai = bi = 0

                def slab_of(h):
                    return {0: 0, 1: 0, 2: 1}[h]

                def run_piece(h, kb0):
                    r = pv_piece(h, qc, kb0, nkb, ptmap)
                    if r is not None:
                        norm_head(h, slab_of(h), qc, r)

                Slots.left = len(h01_pairs) + len(h2_pairs) + 1
                for i, sides in enumerate(h01_pairs):
                    st_pair(sides, qc, ptmap)
                    pop_fillers()
                    Slots.left -= 1
                    if i >= 3 and ai < len(qA):
                        run_piece(*qA[ai]); ai += 1
                for j2, sides in enumerate(h2_pairs):
                    st_pair(sides, qc, ptmap)
                    pop_fillers()
                    Slots.left -= 1
                    if ai < len(qA):
                        run_piece(*qA[ai]); ai += 1
                    elif bi < len(qB) and 2 * bi + 2 <= j2:
                        run_piece(*qB[bi]); bi += 1
                # tail: remaining pieces alternate with leftover fillers
                for h, kb0 in qA[ai:] + qB[bi:]:
                    run_piece(h, kb0)
                    if fillers:
                        fillers.pop(0)()
                Slots.left = 1
                pop_fillers()

            for f in make_proj_fillers(TCH - 1):
                f()

    nc.compile()
    return nc


_NC_CACHE: dict[int, bass.Bass] = {}


def get_nc(seq_len: int) -> bass.Bass:
    if seq_len not in _NC_CACHE:
        _NC_CACHE[seq_len] = build_nc(seq_len)
    return _NC_CACHE[seq_len]


def make_in_maps(x: np.ndarray, w_attn: np.ndarray, w_proj: np.ndarray):
    """Per-core input dicts. Core c: batch c//4, head group c%4 (3 heads)."""
    bf16 = ml_dtypes.bfloat16
    in_maps = []
    for c in range(N_CORES):
        b, hg = divmod(c, 4)
        q = w_attn[192 * hg: 192 * hg + 192]
        k = w_attn[768 + 192 * hg: 768 + 192 * hg + 192]
        v = w_attn[1536 + 192 * hg: 1536 + 192 * hg + 192]
        wqk = np.concatenate([q[0:128], k[0:128], q[128:192], k[128:192]], axis=0)
        in_maps.append({
            "xT": np.ascontiguousarray(x[b].T).astype(bf16),
            "wqkT": np.ascontiguousarray(wqk.T).astype(bf16),
            "wvT": np.ascontiguousarray(v.T).astype(bf16),
            "wpT": np.ascontiguousarray(
                w_proj[:, 192 * hg: 192 * hg + 192].T
            ).astype(bf16),
        })
    return in_maps


def run_on_cores(x, w_attn, w_proj, trace: bool = False):
    from concourse.bass_utils import run_bass_kernel_spmd

    x = np.asarray(x, dtype=np.float32)
    w_attn = np.asarray(w_attn, dtype=np.float32)
    w_proj = np.asarray(w_proj, dtype=np.float32)
    nc = get_nc(x.shape[1])
    in_maps = make_in_maps(x, w_attn, w_proj)
    res = run_bass_kernel_spmd(
        nc, in_maps, core_ids=list(range(N_CORES)), trace=trace
    )
    outs = [r["out"] for r in res.results]
    full = np.stack(
        [sum(outs[4 * b + hg] for hg in range(4)) for b in range(B)], axis=0
    )
    return full, res


def kernel(x, w_attn, w_proj):
    full, _ = run_on_cores(x, w_attn, w_proj, trace=False)
    return full
